# revision 11
# baseline (speedup 1.0000x reference)
"""Dice + contrastive loss on 8 Trainium2 NeuronCores.

Sharding: every input tensor [16,1,512,512] is flattened to [16, 262144]
and sharded along the *pixel* axis (32768 pixels per core).  With that
split every term of the loss becomes a local partial reduction:

  - dice:   sum(sigmoid(pred)), sum(sigmoid(pred)*gt), sum(gt)  (scalars)
  - pos:    sum((mask*(s1-s2))^2) per image              (diag of a Gram)
  - sq1/sq2: sum(s1^2), sum(s2^2) per image              (diag of a Gram)
  - cross:  s1 @ s2.T (16x16 Gram), contraction over pixels

Per-core layout: [128 partitions, 16 img x 256] with Gram-pack columns
col = t*128 + s*16 + b (t of 32 contraction chunks, s of 8 sub-cols).

Engine assignment (the Act engine's 3 sigmoid streams are the compute
roofline at the measured ~1.1 ns/col; everything else hides under it):
  - Act:  sigmoid(in1/in2) in interleaved quarter chunks, sigmoid(pred)
          last in shrinking chunks (so the trailing p*gt pass is short);
          pred chunks carry accum_out -> sum_p.  A dummy 1-col sigmoid
          at the queue head pulls ACT_TABLE_LOAD into the DMA window.
  - DVE:  d = s1-s2 and dm = d*mask as 2x-mode tensor_tensor, fused
          scalar_tensor_tensor p*gt chunks with accum_out -> sum_pg
          trailing the pred sigmoids, PSUM evacuations in the gaps.
  - PE:   Gram A (s1 stationary, [s1|s2] moving -> sq1+cross), B (s2 ->
          sq2), C (dm -> pos), D (ones stationary, gt moving -> sum_g
          column sums).  D runs on the early-arriving gt pieces - that
          also ramps the PE p-state - and stops early so its result
          ships mid-kernel.
  - DMA:  triggers cost ~0.7us of queue time each plus ~3.6us of
          descriptor-pipeline latency, so input triggers are split
          between the Sync and Pool queues, the first in1/in2 pieces
          are small to start Act sooner, and all [P,*] outputs merge
          into ONE DMA (gA|gB|gC|stats) so the tail pays the trigger
          latency once.

dtypes: in1/in2/pred/gt ship as fp8 e4m3 (sums/products of 262144
random-rounded values keep relative error ~1e-4), mask as bf16 so the
d*mask pass runs in DVE 2x mode.  3.0 MiB of input DMA per core.
The tiny cross-core combine (a few KiB per core) happens on the host.
"""

import os
import sys

sys.path.insert(0, "/opt/trn_rl_repo")

import numpy as np
import ml_dtypes

import concourse.bass as bass
import concourse.tile as tile
from concourse import bacc, mybir
from concourse.bass_utils import run_bass_kernel_spmd

TAU = 0.1
DICE_SMOOTH = 0.1
WEIGHT = 1.0

NCORES = 8
B = 16                      # batch (images)
NPIX = 512 * 512            # pixels per image
PIX = NPIX // NCORES        # pixels per image per core = 32768
P = 128                     # partitions
F = PIX // P                # free columns per image per core = 256
T = 32                      # Gram contraction chunks (each covers 8 f-columns)
S = F // T                  # sub-columns per chunk = 8
NC = B * F                  # total free columns per tensor per core = 4096
Q = 4                       # Act/DVE quarter chunks for s1/s2/d/dm
QC = NC // Q                # columns per quarter = 1024
TQ = T // Q                 # t-chunks per quarter = 8
# pred/p*gt processed in shrinking chunks so the trailing stt is short
PCH = [1024, 1024, 1024, 512, 512]
NST = len(PCH)
NOUT = 2 * P + 2 * P + 2 * NST      # merged output cols: A|B|C|stats

F32 = mybir.dt.float32
BF16 = mybir.dt.bfloat16
FP8 = mybir.dt.float8e4
NP_BF16 = ml_dtypes.bfloat16
NP_FP8 = ml_dtypes.float8_e4m3
AF = mybir.ActivationFunctionType
ALU = mybir.AluOpType
AX = mybir.AxisListType


def _build_program():
    nc = bacc.Bacc("TRN2", target_bir_lowering=False, debug=False,
                   num_devices=NCORES)

    # ---- DRAM I/O (per-core shapes), Gram-pack layout col=(t,s,b) ----
    d_in1 = nc.dram_tensor("in1", [P, NC], FP8, kind="ExternalInput")
    d_in2 = nc.dram_tensor("in2", [P, NC], FP8, kind="ExternalInput")
    d_pred = nc.dram_tensor("pred", [P, NC], FP8, kind="ExternalInput")
    d_mask = nc.dram_tensor("mask", [P, NC], BF16, kind="ExternalInput")
    d_gt = nc.dram_tensor("gt", [P, NC], FP8, kind="ExternalInput")

    o_all = nc.dram_tensor("allout", [P, NOUT], F32, kind="ExternalOutput")
    o_gD = nc.dram_tensor("gD", [1, 512], F32, kind="ExternalOutput")

    with tile.TileContext(nc) as tc:
        with tc.tile_pool(name="main", bufs=1) as pool:
            t_in1 = pool.tile([P, NC], FP8, name="t_in1", tag="t_in1")
            t_in2 = pool.tile([P, NC], FP8, name="t_in2", tag="t_in2")
            t_pred = pool.tile([P, NC], FP8, name="t_pred", tag="t_pred")
            t_mask = pool.tile([P, NC], BF16, name="t_mask", tag="t_mask")
            t_gt = pool.tile([P, NC], FP8, name="t_gt", tag="t_gt")
            # s12: col = t*256 + h*128 + (s*16+b), h=0: s1, h=1: s2
            s12 = pool.tile([P, 2 * NC], BF16, name="s12", tag="s12")
            # dd: h=0: d = s1-s2, h=1: dm = d*mask
            dd = pool.tile([P, 2 * NC], BF16, name="dd", tag="dd")
            t_p = pool.tile([P, NC], BF16, name="t_p", tag="t_p")
            scr = pool.tile([P, QC], BF16, name="scr", tag="scr")
            ones8 = pool.tile([P, P], FP8, name="ones8", tag="ones8")
            onesb = pool.tile([P, 1], BF16, name="onesb", tag="onesb")
            allout = pool.tile([P, NOUT], F32, name="allout_sb", tag="allout_sb")
            gD_sb = pool.tile([1, 512], F32, name="gD_sb", tag="gD_sb")

            with tc.tile_pool(name="psum", bufs=1, space="PSUM") as psum_pool:
                psA = psum_pool.tile([P, 2 * P], F32, name="psA", tag="psA")
                psB = psum_pool.tile([P, P], F32, name="psB", tag="psB")
                psC = psum_pool.tile([P, P], F32, name="psC", tag="psC")
                psD = psum_pool.tile([P, 512], F32, name="psD", tag="psD")

                v_s12 = s12[:].rearrange("p (t h c) -> p t h c", h=2, c=P)
                v_dd = dd[:].rearrange("p (t h c) -> p t h c", h=2, c=P)

                def qsl(q):          # t-chunk slice of quarter q
                    return slice(q * TQ, (q + 1) * TQ)

                def qv(t, q):        # quarter view of a [P, NC] tile
                    return t[:, q * QC:(q + 1) * QC].rearrange(
                        "p (t c) -> p t c", c=P)

                # constants (DVE; cheap, before its compute stream)
                nc.vector.memset(onesb[:], 1.0)
                nc.vector.memset(ones8[:], 1.0)

                # Act: pull the sigmoid table load into the DMA window
                nc.scalar.activation(scr[:, 0:1], onesb[:], AF.Sigmoid)

                # ---- input DMAs: sync + pool queues, piecewise ----
                def dma_in(eng, dram, t, lo, hi):
                    eng.dma_start(t[:, lo:hi], dram.ap()[:, lo:hi])

                dma_in(nc.sync, d_in1, t_in1, 0, QC)        # small first piece
                dma_in(nc.gpsimd, d_in2, t_in2, 0, QC)
                dma_in(nc.sync, d_in1, t_in1, QC, NC)
                dma_in(nc.gpsimd, d_in2, t_in2, QC, NC)
                dma_in(nc.sync, d_mask, t_mask, 0, 2 * QC)
                dma_in(nc.gpsimd, d_gt, t_gt, 0, 2 * QC)
                dma_in(nc.sync, d_mask, t_mask, 2 * QC, NC)
                dma_in(nc.gpsimd, d_pred, t_pred, 0, 2 * QC)
                dma_in(nc.gpsimd, d_pred, t_pred, 2 * QC, NC)
                dma_in(nc.gpsimd, d_gt, t_gt, 2 * QC, NC)

                # ---- Act: sigmoids, s1/s2 interleaved by quarter ----
                for q in range(Q):
                    nc.scalar.activation(v_s12[:, qsl(q), 0, :],
                                         qv(t_in1, q), AF.Sigmoid)
                    nc.scalar.activation(v_s12[:, qsl(q), 1, :],
                                         qv(t_in2, q), AF.Sigmoid)
                off = 0
                for i, w in enumerate(PCH):
                    nc.scalar.activation(t_p[:, off:off + w],
                                         t_pred[:, off:off + w], AF.Sigmoid,
                                         accum_out=allout[:, 4 * P + i:4 * P + i + 1])
                    off += w

                # ---- DVE: d = s1-s2, dm = d*mask (2x mode) ----
                for q in range(Q):
                    nc.vector.tensor_tensor(v_dd[:, qsl(q), 0, :],
                                            v_s12[:, qsl(q), 0, :],
                                            v_s12[:, qsl(q), 1, :],
                                            ALU.subtract)
                    nc.vector.tensor_tensor(v_dd[:, qsl(q), 1, :],
                                            v_dd[:, qsl(q), 0, :],
                                            qv(t_mask, q), ALU.mult)

                def stt(i, off, w):
                    c = 4 * P + NST + i
                    nc.vector.scalar_tensor_tensor(
                        scr[:, 0:w], t_p[:, off:off + w], 1.0,
                        t_gt[:, off:off + w], ALU.mult, ALU.mult,
                        accum_out=allout[:, c:c + 1])

                # ---- PE: Grams (PSUM-accumulated over all 32 t-chunks) ----
                s12r = s12[:]
                ddr = dd[:]

                def d_block(g):      # 2 x 4 chunks of 512 gt cols
                    for k in range(4 * g, 4 * (g + 1)):
                        st = dict(start=(k == 0), stop=(k == 7))
                        nc.tensor.matmul(psD[:], ones8[:],
                                         t_gt[:, k * 512:(k + 1) * 512],
                                         **st)

                def ab_block(q):
                    for t in range(q * TQ, (q + 1) * TQ):
                        st = dict(start=(t == 0), stop=(t == T - 1))
                        c0, c1, c2 = t * 2 * P, t * 2 * P + P, (t + 1) * 2 * P
                        nc.tensor.matmul(psA[:], s12r[:, c0:c1],
                                         s12r[:, c0:c2], **st)
                        nc.tensor.matmul(psB[:], s12r[:, c1:c2],
                                         s12r[:, c1:c2], **st)

                def c_block(q):
                    for t in range(q * TQ, (q + 1) * TQ):
                        st = dict(start=(t == 0), stop=(t == T - 1))
                        c1, c2 = t * 2 * P + P, (t + 1) * 2 * P
                        nc.tensor.matmul(psC[:], ddr[:, c1:c2],
                                         ddr[:, c1:c2], **st)

                d_block(0)
                ab_block(0)
                c_block(0)
                d_block(1)

                # gD evacuation (psD stopped early); DVE reaches this
                # between its dm and stt streams, then ships it mid-kernel
                nc.vector.tensor_copy(gD_sb[:], psD[0:1, :])
                nc.sync.dma_start(o_gD.ap(), gD_sb[:])

                off = 0
                for i, w in enumerate(PCH[:3]):
                    stt(i, off, w)
                    off += w

                ab_block(1)
                c_block(1)
                ab_block(2)
                c_block(2)
                ab_block(3)
                c_block(3)

                # ---- evacuate PSUM -> SBUF (DVE), tail stt, outputs ----
                nc.vector.tensor_copy(allout[:, 0:2 * P], psA[:])
                nc.vector.tensor_copy(allout[:, 2 * P:3 * P], psB[:])
                nc.vector.tensor_copy(allout[:, 3 * P:4 * P], psC[:])

                off = sum(PCH[:3])
                for i in range(3, NST):
                    stt(i, off, PCH[i])
                    off += PCH[i]

                nc.sync.dma_start(o_all.ap(), allout[:])

    nc.compile()
    return nc


_NC_CACHE = None


def _get_program():
    global _NC_CACHE
    if _NC_CACHE is None:
        _NC_CACHE = _build_program()
    return _NC_CACHE


def _shard_inputs(pred_labeled, gt_labeled, input1, input2, mask):
    flat = {
        "pred": np.asarray(pred_labeled, dtype=np.float32).reshape(B, NPIX),
        "gt": np.asarray(gt_labeled, dtype=np.float32).reshape(B, NPIX),
        "in1": np.asarray(input1, dtype=np.float32).reshape(B, NPIX),
        "in2": np.asarray(input2, dtype=np.float32).reshape(B, NPIX),
        "mask": np.asarray(mask, dtype=np.float32).reshape(B, NPIX),
    }

    def pack(a, sl, dt):  # Gram pack: [P, (t s b)]
        return np.ascontiguousarray(
            a[:, sl].reshape(B, P, T, S).transpose(1, 2, 3, 0)
            .reshape(P, NC)).astype(dt)

    in_maps = []
    for k in range(NCORES):
        sl = slice(k * PIX, (k + 1) * PIX)
        in_maps.append({
            "in1": pack(flat["in1"], sl, NP_FP8),
            "in2": pack(flat["in2"], sl, NP_FP8),
            "pred": pack(flat["pred"], sl, NP_FP8),
            "mask": pack(flat["mask"], sl, NP_BF16),
            "gt": pack(flat["gt"], sl, NP_FP8)})
    return in_maps


def _block_diag_sum(gmat):
    # [128, 128] with rows (s*16+b1), cols (s*16+b2) -> sum_s of [16,16] blocks
    g = gmat.reshape(S, B, S, B)
    return np.einsum("sbsc->bc", g)


def _combine(results):
    sum_p = sum_pg = sum_g = 0.0
    g1 = np.zeros((B, B), np.float64)
    cr = np.zeros((B, B), np.float64)
    g2 = np.zeros((B, B), np.float64)
    pc = np.zeros((B, B), np.float64)
    for r in results:
        ao = r["allout"].astype(np.float64)
        sum_p += ao[:, 4 * P:4 * P + NST].sum()
        sum_pg += ao[:, 4 * P + NST:4 * P + 2 * NST].sum()
        sum_g += r["gD"].astype(np.float64).sum()
        g1 += _block_diag_sum(ao[:, :P])
        cr += _block_diag_sum(ao[:, P:2 * P])
        g2 += _block_diag_sum(ao[:, 2 * P:3 * P])
        pc += _block_diag_sum(ao[:, 3 * P:4 * P])

    dice = 1.0 - (2.0 * sum_pg + DICE_SMOOTH) / (sum_p + sum_g + DICE_SMOOTH)

    n = float(NPIX)
    sq1 = np.diag(g1) / n
    sq2 = np.diag(g2) / n
    cross = cr / n
    pos_mse = np.diag(pc) / n

    sim_pos = np.exp(-pos_mse / TAU)
    mse = sq1[:, None] + sq2[None, :] - 2.0 * cross
    sim = np.exp(-mse / TAU)
    sim_neg = (sim * (1.0 - np.eye(B))).sum(axis=1)
    loss_c = float(np.mean(-np.log(sim_pos / (sim_pos + sim_neg))))
    total = dice + WEIGHT * loss_c
    return (np.float32(total), np.float32(dice), 0.0, np.float32(loss_c))


def kernel(pred_labeled, gt_labeled, input1, input2, mask):
    nc = _get_program()
    in_maps = _shard_inputs(pred_labeled, gt_labeled, input1, input2, mask)
    res = run_bass_kernel_spmd(nc, in_maps, core_ids=list(range(NCORES)),
                               trace=bool(int(os.environ.get("KERNEL_TRACE", "0"))))
    out = _combine(res.results)
    if res.exec_time_ns is not None:
        print(f"HW exec time: {res.exec_time_ns} ns")
    return out


# revision 12
# speedup vs baseline: 1.0787x; 1.0787x over previous
"""Dice + contrastive loss on 8 Trainium2 NeuronCores.

Sharding: every input tensor [16,1,512,512] is flattened to [16, 262144]
and sharded along the *pixel* axis (32768 pixels per core).  With that
split every term of the loss becomes a local partial reduction:

  - dice:   sum(sigmoid(pred)), sum(sigmoid(pred)*gt), sum(gt)  (scalars)
  - pos:    sum((mask*(s1-s2))^2) per image              (diag of a Gram)
  - sq1/sq2: sum(s1^2), sum(s2^2) per image              (diag of a Gram)
  - cross:  s1 @ s2.T (16x16 Gram), contraction over pixels

Per-core layout: [128 partitions, 16 img x 256] with Gram-pack columns
col = t*128 + s*16 + b (t of 32 contraction chunks, s of 8 sub-cols).

The profile's exec-time metric spans first-useful-instruction (the
first DMA trigger, ~6-7us after NEFF start) to last-useful-end (the
final output DMA packet), so the design minimizes the critical path
from trigger to final DMA:

  - Act:  sigmoid(in1/in2) in interleaved quarter chunks, sigmoid(pred)
          in 3 chunks carrying accum_out -> sum_p, then the A/B/C PSUM
          evacuations (Act is idle by then; DVE is not).  A dummy 1-col
          sigmoid pulls ACT_TABLE_LOAD into the DMA-fill window.
  - DVE:  d = s1-s2 and dm = d*mask as 2x-mode tensor_tensor, the
          psD row reduction -> sum_g scalar, and fused
          scalar_tensor_tensor p*gt chunks (accum_out -> sum_pg)
          trailing the pred sigmoids.
  - PE:   Gram A (s1 stationary, [s1|s2] moving -> sq1+cross), B (s2 ->
          sq2), C (dm -> pos), D (ones stationary, gt moving -> sum_g
          column sums), emission-ordered so the queue head never waits
          on late-arriving data.
  - DMA:  inputs all fp8 (2.5 MiB/core; ~280 GB/s/core is the shared-
          engine limit with 8 cores pulling at once) split between the
          Sync and Pool trigger queues; every [P,*] result plus the
          scalar accumulators merges into ONE output tensor so the tail
          pays the trigger + descriptor-pipeline latency exactly once.

dtypes: all five inputs ship as fp8 e4m3 - sums/products of 262144
random-rounded values keep relative error ~1e-4 (verified 5e-4 vs the
f32 reference end-to-end).  The d*mask and p*gt passes read one fp8
operand so they run at DVE 1x rate; d = s1-s2 runs at 2x.
The tiny cross-core combine (a few KiB per core) happens on the host.
"""

import os
import sys

sys.path.insert(0, "/opt/trn_rl_repo")

import numpy as np
import ml_dtypes

import concourse.bass as bass
import concourse.tile as tile
from concourse import bacc, mybir
from concourse.bass_utils import run_bass_kernel_spmd

TAU = 0.1
DICE_SMOOTH = 0.1
WEIGHT = 1.0

NCORES = 8
B = 16                      # batch (images)
NPIX = 512 * 512            # pixels per image
PIX = NPIX // NCORES        # pixels per image per core = 32768
P = 128                     # partitions
F = PIX // P                # free columns per image per core = 256
T = 32                      # Gram contraction chunks (each covers 8 f-columns)
S = F // T                  # sub-columns per chunk = 8
NC = B * F                  # total free columns per tensor per core = 4096
Q = 4                       # Act/DVE quarter chunks for s1/s2/d/dm
QC = NC // Q                # columns per quarter = 1024
TQ = T // Q                 # t-chunks per quarter = 8
PCH = [2048, 1024, 1024]    # pred sigmoid / p*gt chunks
NST = len(PCH)
# merged output columns: A | B | C | sum_p x3 | sum_pg x3 | sum_g | pad
OFF_A, OFF_B, OFF_C = 0, 2 * P, 3 * P
OFF_SP, OFF_SPG = 4 * P, 4 * P + NST
OFF_SG = 4 * P + 2 * NST
NOUT = OFF_SG + 2

F32 = mybir.dt.float32
BF16 = mybir.dt.bfloat16
FP8 = mybir.dt.float8e4
NP_BF16 = ml_dtypes.bfloat16
NP_FP8 = ml_dtypes.float8_e4m3
AF = mybir.ActivationFunctionType
ALU = mybir.AluOpType
AX = mybir.AxisListType


def _build_program():
    nc = bacc.Bacc("TRN2", target_bir_lowering=False, debug=False,
                   num_devices=NCORES)

    # ---- DRAM I/O (per-core shapes), Gram-pack layout col=(t,s,b) ----
    d_in1 = nc.dram_tensor("in1", [P, NC], FP8, kind="ExternalInput")
    d_in2 = nc.dram_tensor("in2", [P, NC], FP8, kind="ExternalInput")
    d_pred = nc.dram_tensor("pred", [P, NC], FP8, kind="ExternalInput")
    d_mask = nc.dram_tensor("mask", [P, NC], FP8, kind="ExternalInput")
    d_gt = nc.dram_tensor("gt", [P, NC], FP8, kind="ExternalInput")

    o_all = nc.dram_tensor("allout", [P, NOUT], F32, kind="ExternalOutput")

    with tile.TileContext(nc) as tc:
        with tc.tile_pool(name="main", bufs=1) as pool:
            t_in1 = pool.tile([P, NC], FP8, name="t_in1", tag="t_in1")
            t_in2 = pool.tile([P, NC], FP8, name="t_in2", tag="t_in2")
            t_pred = pool.tile([P, NC], FP8, name="t_pred", tag="t_pred")
            t_mask = pool.tile([P, NC], FP8, name="t_mask", tag="t_mask")
            t_gt = pool.tile([P, NC], FP8, name="t_gt", tag="t_gt")
            # s12: col = t*256 + h*128 + (s*16+b), h=0: s1, h=1: s2
            s12 = pool.tile([P, 2 * NC], BF16, name="s12", tag="s12")
            # dd: h=0: d = s1-s2, h=1: dm = d*mask
            dd = pool.tile([P, 2 * NC], BF16, name="dd", tag="dd")
            t_p = pool.tile([P, NC], BF16, name="t_p", tag="t_p")
            scr = pool.tile([P, 2 * QC], BF16, name="scr", tag="scr")
            ones8 = pool.tile([P, P], FP8, name="ones8", tag="ones8")
            onesb = pool.tile([P, 1], BF16, name="onesb", tag="onesb")
            allout = pool.tile([P, NOUT], F32, name="allout_sb", tag="allout_sb")

            with tc.tile_pool(name="psum", bufs=1, space="PSUM") as psum_pool:
                psA = psum_pool.tile([P, 2 * P], F32, name="psA", tag="psA")
                psB = psum_pool.tile([P, P], F32, name="psB", tag="psB")
                psC = psum_pool.tile([P, P], F32, name="psC", tag="psC")
                psD = psum_pool.tile([P, 512], F32, name="psD", tag="psD")

                v_s12 = s12[:].rearrange("p (t h c) -> p t h c", h=2, c=P)
                v_dd = dd[:].rearrange("p (t h c) -> p t h c", h=2, c=P)

                def qsl(q):          # t-chunk slice of quarter q
                    return slice(q * TQ, (q + 1) * TQ)

                def qv(t, q):        # quarter view of a [P, NC] tile
                    return t[:, q * QC:(q + 1) * QC].rearrange(
                        "p (t c) -> p t c", c=P)

                # constants (DVE queue; lands with/after the first trigger)
                nc.vector.memset(onesb[:], 1.0)
                nc.vector.memset(ones8[:], 1.0)

                # Act: pull the sigmoid table load into the DMA window
                nc.scalar.activation(scr[:, 0:1], onesb[:], AF.Sigmoid)

                # ---- input DMAs: sync + pool queues, piecewise ----
                def dma_in(eng, dram, t, lo, hi):
                    eng.dma_start(t[:, lo:hi], dram.ap()[:, lo:hi])

                dma_in(nc.sync, d_in1, t_in1, 0, QC)        # small first piece
                dma_in(nc.gpsimd, d_in2, t_in2, 0, QC)
                dma_in(nc.sync, d_in1, t_in1, QC, NC)
                dma_in(nc.gpsimd, d_in2, t_in2, QC, NC)
                dma_in(nc.sync, d_mask, t_mask, 0, NC)
                dma_in(nc.gpsimd, d_gt, t_gt, 0, 2 * QC)
                dma_in(nc.gpsimd, d_pred, t_pred, 0, 2 * QC)
                dma_in(nc.gpsimd, d_pred, t_pred, 2 * QC, NC)
                dma_in(nc.gpsimd, d_gt, t_gt, 2 * QC, NC)

                # ---- Act: sigmoids, s1/s2 interleaved by quarter ----
                for q in range(Q):
                    nc.scalar.activation(v_s12[:, qsl(q), 0, :],
                                         qv(t_in1, q), AF.Sigmoid)
                    nc.scalar.activation(v_s12[:, qsl(q), 1, :],
                                         qv(t_in2, q), AF.Sigmoid)
                off = 0
                for i, w in enumerate(PCH):
                    nc.scalar.activation(t_p[:, off:off + w],
                                         t_pred[:, off:off + w], AF.Sigmoid,
                                         accum_out=allout[:, OFF_SP + i:OFF_SP + i + 1])
                    off += w

                # ---- DVE: d = s1-s2 (2x), dm = d*mask (1x) ----
                for q in range(Q):
                    nc.vector.tensor_tensor(v_dd[:, qsl(q), 0, :],
                                            v_s12[:, qsl(q), 0, :],
                                            v_s12[:, qsl(q), 1, :],
                                            ALU.subtract)
                    nc.vector.tensor_tensor(v_dd[:, qsl(q), 1, :],
                                            v_dd[:, qsl(q), 0, :],
                                            qv(t_mask, q), ALU.mult)

                # ---- PE: Grams (PSUM-accumulated over all 32 t-chunks) ----
                s12r = s12[:]
                ddr = dd[:]

                def d_block(g):      # 2 x 4 chunks of 512 gt cols
                    for k in range(4 * g, 4 * (g + 1)):
                        st = dict(start=(k == 0), stop=(k == 7))
                        nc.tensor.matmul(psD[:], ones8[:],
                                         t_gt[:, k * 512:(k + 1) * 512],
                                         **st)

                def ab_block(q):
                    for t in range(q * TQ, (q + 1) * TQ):
                        st = dict(start=(t == 0), stop=(t == T - 1))
                        c0, c1, c2 = t * 2 * P, t * 2 * P + P, (t + 1) * 2 * P
                        nc.tensor.matmul(psA[:], s12r[:, c0:c1],
                                         s12r[:, c0:c2], **st)
                        nc.tensor.matmul(psB[:], s12r[:, c1:c2],
                                         s12r[:, c1:c2], **st)

                def c_block(q):
                    for t in range(q * TQ, (q + 1) * TQ):
                        st = dict(start=(t == 0), stop=(t == T - 1))
                        c1, c2 = t * 2 * P + P, (t + 1) * 2 * P
                        nc.tensor.matmul(psC[:], ddr[:, c1:c2],
                                         ddr[:, c1:c2], **st)

                ab_block(0)
                d_block(0)
                ab_block(1)
                d_block(1)
                ab_block(2)
                ab_block(3)
                c_block(0)
                c_block(1)
                c_block(2)
                c_block(3)

                # sum_g: reduce psD row 0 to one scalar (DVE, runs early)
                nc.vector.tensor_reduce(allout[0:1, OFF_SG:OFF_SG + 1],
                                        psD[0:1, :], axis=AX.X, op=ALU.add)

                # ---- DVE: fused p*gt chunks trailing the pred sigmoids ----
                off = 0
                for i, w in enumerate(PCH):
                    nc.vector.scalar_tensor_tensor(
                        scr[:, 0:w], t_p[:, off:off + w], 1.0,
                        t_gt[:, off:off + w], ALU.mult, ALU.mult,
                        accum_out=allout[:, OFF_SPG + i:OFF_SPG + i + 1])
                    off += w

                # ---- evacuate A/B/C PSUM -> SBUF on the idle Act queue ----
                nc.scalar.copy(allout[:, OFF_A:OFF_A + 2 * P], psA[:])
                nc.scalar.copy(allout[:, OFF_B:OFF_B + P], psB[:])
                nc.scalar.copy(allout[:, OFF_C:OFF_C + P], psC[:])

                nc.sync.dma_start(o_all.ap(), allout[:])

    nc.compile()
    return nc


_NC_CACHE = None


def _get_program():
    global _NC_CACHE
    if _NC_CACHE is None:
        _NC_CACHE = _build_program()
    return _NC_CACHE


def _shard_inputs(pred_labeled, gt_labeled, input1, input2, mask):
    flat = {
        "pred": np.asarray(pred_labeled, dtype=np.float32).reshape(B, NPIX),
        "gt": np.asarray(gt_labeled, dtype=np.float32).reshape(B, NPIX),
        "in1": np.asarray(input1, dtype=np.float32).reshape(B, NPIX),
        "in2": np.asarray(input2, dtype=np.float32).reshape(B, NPIX),
        "mask": np.asarray(mask, dtype=np.float32).reshape(B, NPIX),
    }

    def pack(a, sl, dt):  # Gram pack: [P, (t s b)]
        return np.ascontiguousarray(
            a[:, sl].reshape(B, P, T, S).transpose(1, 2, 3, 0)
            .reshape(P, NC)).astype(dt)

    in_maps = []
    for k in range(NCORES):
        sl = slice(k * PIX, (k + 1) * PIX)
        in_maps.append({
            "in1": pack(flat["in1"], sl, NP_FP8),
            "in2": pack(flat["in2"], sl, NP_FP8),
            "pred": pack(flat["pred"], sl, NP_FP8),
            "mask": pack(flat["mask"], sl, NP_FP8),
            "gt": pack(flat["gt"], sl, NP_FP8)})
    return in_maps


def _block_diag_sum(gmat):
    # [128, 128] with rows (s*16+b1), cols (s*16+b2) -> sum_s of [16,16] blocks
    g = gmat.reshape(S, B, S, B)
    return np.einsum("sbsc->bc", g)


def _combine(results):
    sum_p = sum_pg = sum_g = 0.0
    g1 = np.zeros((B, B), np.float64)
    cr = np.zeros((B, B), np.float64)
    g2 = np.zeros((B, B), np.float64)
    pc = np.zeros((B, B), np.float64)
    for r in results:
        ao = r["allout"].astype(np.float64)
        sum_p += ao[:, OFF_SP:OFF_SP + NST].sum()
        sum_pg += ao[:, OFF_SPG:OFF_SPG + NST].sum()
        sum_g += ao[0, OFF_SG]
        g1 += _block_diag_sum(ao[:, OFF_A:OFF_A + P])
        cr += _block_diag_sum(ao[:, OFF_A + P:OFF_A + 2 * P])
        g2 += _block_diag_sum(ao[:, OFF_B:OFF_B + P])
        pc += _block_diag_sum(ao[:, OFF_C:OFF_C + P])

    dice = 1.0 - (2.0 * sum_pg + DICE_SMOOTH) / (sum_p + sum_g + DICE_SMOOTH)

    n = float(NPIX)
    sq1 = np.diag(g1) / n
    sq2 = np.diag(g2) / n
    cross = cr / n
    pos_mse = np.diag(pc) / n

    sim_pos = np.exp(-pos_mse / TAU)
    mse = sq1[:, None] + sq2[None, :] - 2.0 * cross
    sim = np.exp(-mse / TAU)
    sim_neg = (sim * (1.0 - np.eye(B))).sum(axis=1)
    loss_c = float(np.mean(-np.log(sim_pos / (sim_pos + sim_neg))))
    total = dice + WEIGHT * loss_c
    return (np.float32(total), np.float32(dice), 0.0, np.float32(loss_c))


def kernel(pred_labeled, gt_labeled, input1, input2, mask):
    nc = _get_program()
    in_maps = _shard_inputs(pred_labeled, gt_labeled, input1, input2, mask)
    res = run_bass_kernel_spmd(nc, in_maps, core_ids=list(range(NCORES)),
                               trace=bool(int(os.environ.get("KERNEL_TRACE", "0"))))
    out = _combine(res.results)
    if res.exec_time_ns is not None:
        print(f"HW exec time: {res.exec_time_ns} ns")
    return out
